# revision 67
# baseline (speedup 1.0000x reference)
"""Trainium2 Bass kernel: 5x5 window median+variance denoise filter.

y = relu(x - noise_var/(var5x5(x)+1e-10) * (x - median5x5(x) + noise_bias))
with zero-padded 5x5 windows, unbiased variance (ddof=1).

Sharding: pure data parallel, B=16 images split 2-per-core across 8 cores.

Median via a pruned comparator network with shared column sorts:
  sort5 over the 5 dy-shifted planes (9 CE, shared by 5 horizontal windows)
  T = odd-even merge of adjacent sorted columns (13 CE, shared by 2 windows)
  final rank-12 selection from T(x-2), T(x), S(x+2) (35 CE, single-sided
  min/max pruned) -- 90 DVE min/max ops per full-image sweep, verified
  offline by exhaustive 0-1 principle.
"""
import numpy as np

import concourse.bass as bass  # noqa: F401
import concourse.mybir as mybir
from concourse import bacc, tile
from concourse.bass_utils import run_bass_kernel_spmd

F32 = mybir.dt.float32
FP16 = mybir.dt.float16
ALU = mybir.AluOpType
ACTF = mybir.ActivationFunctionType
MEDIAN_FP16 = True    # fp16 comparator network (first sort layer does the
                      # f32->fp16 cast for free; DVE-legal, ~5e-4 rel err)

# (i, j, need_min, need_max) per structure; designed + 0/1-verified offline.
SORT5 = [(0, 1, 1, 1), (3, 4, 1, 1), (2, 4, 1, 1), (2, 3, 1, 1), (0, 3, 1, 1),
         (0, 2, 1, 1), (1, 4, 1, 1), (1, 3, 1, 1), (1, 2, 1, 1)]
T_CES = [(0, 5, 1, 1), (4, 9, 1, 1), (4, 5, 1, 1), (2, 7, 1, 1), (2, 4, 1, 1),
         (7, 5, 1, 1), (1, 6, 1, 1), (3, 8, 1, 1), (3, 6, 1, 1), (1, 2, 1, 1),
         (3, 4, 1, 1), (6, 7, 1, 1), (8, 5, 1, 1)]
F_CES = [(0, 10, 0, 1), (5, 15, 1, 0), (5, 10, 1, 1), (4, 14, 1, 1),
         (4, 5, 0, 1), (14, 10, 1, 0), (2, 12, 0, 1), (7, 17, 1, 0),
         (7, 12, 1, 1), (7, 5, 0, 1), (12, 14, 1, 1), (1, 11, 0, 1),
         (9, 19, 1, 0), (9, 11, 1, 1), (6, 16, 1, 1), (6, 9, 0, 1),
         (16, 11, 1, 0), (3, 13, 0, 1), (8, 18, 1, 0), (8, 13, 1, 1),
         (8, 9, 1, 1), (13, 16, 1, 0), (8, 5, 1, 1), (9, 12, 1, 1),
         (13, 14, 1, 1), (8, 20, 0, 1), (13, 24, 1, 0), (13, 20, 0, 1),
         (9, 22, 0, 1), (22, 20, 1, 0), (5, 21, 0, 1), (14, 21, 1, 0),
         (12, 23, 1, 0), (12, 14, 0, 1), (14, 22, 1, 0)]
F_OUT = 14

H = 512
W = 512
IMGS_PER_CORE = 2
N_CORES = 8
WIDE = W + 4          # 2-col halo each side
NBUF = 46             # cap on SBUF working buffers of [128, 2, WIDE] f32


class BufPool:
    """Free-list over preallocated fixed SBUF tensors. Tile's dependency
    tracker makes reuse safe (WAR/RAW serialization on the same tensor)."""

    def __init__(self, nc, tag="wb", dtype=F32, cap=NBUF):
        self.nc = nc
        self.tag = tag
        self.dtype = dtype
        self.cap = cap
        self.bufs = []
        self.free = []

    def alloc(self):
        if self.free:
            return self.free.pop()
        idx = len(self.bufs)
        assert idx < self.cap, f"SBUF pool {self.tag} exhausted"
        t = self.nc.alloc_sbuf_tensor(f"{self.tag}{idx}", [128, 2, WIDE],
                                      self.dtype).ap()
        self.bufs.append(t)
        return t

    def release(self, t):
        self.free.append(t)


class Wire:
    """SSA value living at column offset `off` of `buf`."""

    def __init__(self, buf, off, owned, pool, on_die=None):
        self.buf = buf
        self.off = off
        self.owned = owned      # release buf to pool when dead
        self.pool = pool
        self.reads_left = 0
        self.on_die = on_die

    def ap(self, width):
        return self.buf[:, :, self.off:self.off + width]

    def read_done(self):
        self.reads_left -= 1
        if self.reads_left == 0:
            self._die()

    def read_done_zero(self):
        if self.reads_left == 0:
            self._die()

    def _die(self):
        if self.owned:
            self.pool.release(self.buf)
        if self.on_die is not None:
            self.on_die()

    def detach_views(self, n_views):
        """Transfer buffer ownership to n_views future views; returns the
        on_die callback the views share. Call read_done() after to consume
        the terminal hold."""
        buf, owned, pool = self.buf, self.owned, self.pool
        self.owned = False
        state = {"n": n_views}

        def on_die():
            state["n"] -= 1
            if state["n"] == 0 and owned:
                pool.release(buf)
        return on_die


def run_stage(nc, pool, wires, ces, width, terminal_reads, final_pool=None,
              final_wire=None):
    """Emit one structure stage. A position's lifetime is split into segments
    at each rewrite; each Wire object gets the read count of its own segment
    so buffers release as soon as truly dead. The final write of
    `final_wire` (if given) allocates from `final_pool` (dtype switch)."""
    n = len(wires)
    # segs[i] = read counts per segment of position i (segment ends at a
    # write of i, which itself reads the old value).
    segs = [[] for _ in range(n)]
    cur = [0] * n
    last_write = {}
    for ci, (a, b, nmin, nmax) in enumerate(ces):
        cur[a] += 1
        cur[b] += 1
        if nmin:
            segs[a].append(cur[a])
            cur[a] = 0
            last_write[a] = (ci, "min")
        if nmax:
            segs[b].append(cur[b])
            cur[b] = 0
            last_write[b] = (ci, "max")
    for i in range(n):
        segs[i].append(cur[i] + terminal_reads.get(i, 0))

    seg_idx = [0] * n
    for i in range(n):
        wires[i].reads_left += segs[i][0]
        if segs[i][0] == 0:
            wires[i].read_done_zero()

    for ci, (i, j, nmin, nmax) in enumerate(ces):
        wi, wj = wires[i], wires[j]
        a = wi.ap(width)
        b = wj.ap(width)
        if nmin:
            p = (final_pool if final_wire == i
                 and last_write.get(i) == (ci, "min") else pool)
            lo_pool = p
            lo = p.alloc()
            nc.vector.tensor_tensor(lo[:, :, 0:width], a, b, ALU.min)
        if nmax:
            p = (final_pool if final_wire == j
                 and last_write.get(j) == (ci, "max") else pool)
            hi_pool = p
            hi = p.alloc()
            nc.vector.tensor_tensor(hi[:, :, 0:width], a, b, ALU.max)
        wi.read_done()
        wj.read_done()
        if nmin:
            seg_idx[i] += 1
            cnt = segs[i][seg_idx[i]]
            assert cnt > 0, "dead write (should be pruned offline)"
            wires[i] = Wire(lo, 0, True, lo_pool)
            wires[i].reads_left = cnt
        if nmax:
            seg_idx[j] += 1
            cnt = segs[j][seg_idx[j]]
            assert cnt > 0, "dead write (should be pruned offline)"
            wires[j] = Wire(hi, 0, True, hi_pool)
            wires[j].reads_left = cnt


def emit_chunk(nc, pool, hpool, tin, out_tile, xa, ya, nv_ap, nb_ap, img,
               half):
    r0 = half * 256

    # ---- loads: 5 dy-shifted tiles [128, 2, WIDE] from the pre-padded
    # shard (rows/cols already carry the 2-wide zero halo) ----
    for k, dy in enumerate(range(-2, 3)):
        for b in range(2):
            s = img * (H + 4) + r0 + b * 128 + dy + 2
            nc.sync.dma_start(tin[k][:, b, :], xa[s: s + 128, :])

    # ---- variance ----
    full = lambda t: t[:, :, :]
    sum_acc = pool.alloc()
    nc.vector.tensor_tensor(full(sum_acc), full(tin[0]), full(tin[1]), ALU.add)
    for k in (2, 3, 4):
        nc.vector.tensor_tensor(full(sum_acc), full(sum_acc), full(tin[k]),
                                ALU.add)
    sq_acc = pool.alloc()
    nc.scalar.square(full(sq_acc), full(tin[0]))
    sq_tmp = pool.alloc()
    for k in (1, 2, 3, 4):
        nc.scalar.square(full(sq_tmp), full(tin[k]))
        nc.vector.tensor_tensor(full(sq_acc), full(sq_acc), full(sq_tmp),
                                ALU.add)
    pool.release(sq_tmp)

    def hsum(acc):
        o = pool.alloc()
        nc.vector.tensor_tensor(o[:, :, 0:W], acc[:, :, 0:W],
                                acc[:, :, 1:1 + W], ALU.add)
        for k in (2, 3, 4):
            nc.vector.tensor_tensor(o[:, :, 0:W], o[:, :, 0:W],
                                    acc[:, :, k:k + W], ALU.add)
        pool.release(acc)
        return o

    s25 = hsum(sum_acc)
    q25 = hsum(sq_acc)

    d = pool.alloc()
    nc.vector.tensor_tensor(d[:, :, 0:W], s25[:, :, 0:W], s25[:, :, 0:W],
                            ALU.mult)
    pool.release(s25)
    # d = (s25^2 * (-1/25)) + q25
    nc.vector.scalar_tensor_tensor(d[:, :, 0:W], d[:, :, 0:W], -1.0 / 25.0,
                                   q25[:, :, 0:W], ALU.mult, ALU.add)
    pool.release(q25)
    # d = d*(1/24) + 1e-10
    nc.vector.tensor_scalar(d[:, :, 0:W], d[:, :, 0:W], 1.0 / 24.0, 1e-10,
                            ALU.mult, ALU.add)
    rcp = pool.alloc()
    # single-op ~51-ULP reciprocal (HW-verified 3e-6 rel err) vs the 2-op
    # approx_accurate the first version used
    nc.vector.reciprocal_approx_fast(rcp[:, :, 0:W], d[:, :, 0:W])
    pool.release(d)

    # ---- median network (fp16 when MEDIAN_FP16: the first sort layer
    # reads the f32 tiles and writes fp16; final F op emits f32) ----
    np_ = hpool if MEDIAN_FP16 else pool
    s_wires = [Wire(tin[k], 0, False, pool) for k in range(5)]
    run_stage(nc, np_, s_wires, SORT5, WIDE, {k: 1 for k in range(5)})

    t_wires = [None] * 10
    c_views = [None] * 5
    for k in range(5):
        rk = s_wires[k]
        od = rk.detach_views(3)
        t_wires[k] = Wire(rk.buf, rk.off + 0, False, np_, on_die=od)
        t_wires[k + 5] = Wire(rk.buf, rk.off + 1, False, np_, on_die=od)
        c_views[k] = Wire(rk.buf, rk.off + 4, False, np_, on_die=od)
        rk.read_done()      # consume terminal hold

    run_stage(nc, np_, t_wires, T_CES, W + 3, {j: 1 for j in range(10)})

    f_wires = [None] * 25
    for j in range(10):
        tw = t_wires[j]
        od = tw.detach_views(2)
        f_wires[j] = Wire(tw.buf, tw.off + 0, False, np_, on_die=od)
        f_wires[j + 10] = Wire(tw.buf, tw.off + 2, False, np_, on_die=od)
        tw.read_done()
    for k in range(5):
        f_wires[20 + k] = c_views[k]

    run_stage(nc, np_, f_wires, F_CES, W, {F_OUT: 1},
              final_pool=pool, final_wire=F_OUT)
    mid = f_wires[F_OUT]

    # ---- formula: y = relu(x - nv*rcp*((x + nb) - mid)) ----
    xc = tin[2][:, :, 2:2 + W]              # center plane = x
    u = pool.alloc()
    nc.vector.scalar_tensor_tensor(u[:, :, 0:W], xc, nb_ap, mid.ap(W),
                                   ALU.add, ALU.subtract)
    mid.read_done()
    nc.vector.tensor_tensor(u[:, :, 0:W], rcp[:, :, 0:W], u[:, :, 0:W],
                            ALU.mult)
    pool.release(rcp)
    nc.vector.tensor_scalar(u[:, :, 0:W], u[:, :, 0:W], nv_ap, None, ALU.mult)
    nc.vector.tensor_tensor(u[:, :, 0:W], xc, u[:, :, 0:W], ALU.subtract)
    nc.scalar.activation(out_tile[:, :, :], u[:, :, 0:W], ACTF.Relu)
    pool.release(u)

    # ---- store ----
    for b in range(2):
        nc.sync.dma_start(
            ya[img * H + r0 + b * 128: img * H + r0 + b * 128 + 128, :],
            out_tile[:, b, :],
        )


def build_module(repeat=1, hw_loop=None):
    nc = bacc.Bacc(
        "TRN2",
        target_bir_lowering=False,
        debug=False,
        enable_asserts=False,
        num_devices=N_CORES,
    )
    x = nc.dram_tensor("x", [IMGS_PER_CORE, H + 4, WIDE], F32,
                       kind="ExternalInput")
    nvb = nc.dram_tensor("nvb", [128, 2], F32, kind="ExternalInput")
    y = nc.dram_tensor("y", [IMGS_PER_CORE, H, W], F32, kind="ExternalOutput")

    xa = x.ap().flatten_outer_dims()    # [2*516, 516]
    ya = y.ap().flatten_outer_dims()

    with tile.TileContext(nc) as tc:
        pool = BufPool(nc, "wb", F32, 18 if MEDIAN_FP16 else NBUF)
        hpool = BufPool(nc, "wh", FP16, 40)
        nvb_t = nc.alloc_sbuf_tensor("nvb_t", [128, 2], F32).ap()
        nc.sync.dma_start(nvb_t[:, :], nvb.ap()[:, :])
        nv_ap = nvb_t[:, 0:1]
        nb_ap = nvb_t[:, 1:2]

        tin = [nc.alloc_sbuf_tensor(f"tin{k}", [128, 2, WIDE], F32).ap()
               for k in range(5)]
        out_t = [nc.alloc_sbuf_tensor(f"out{k}", [128, 2, W], F32).ap()
                 for k in range(2)]

        def body():
            for _ in range(repeat):
                for img in range(IMGS_PER_CORE):
                    for half in range(2):
                        emit_chunk(nc, pool, hpool, tin, out_t[half], xa, ya,
                                   nv_ap, nb_ap, img, half)

        if hw_loop is None:
            body()
        else:
            with tc.For_i(0, hw_loop, 1):
                body()

    nc.compile()
    return nc


_MODULE = None


def _get_module():
    global _MODULE
    if _MODULE is None:
        _MODULE = build_module()
    return _MODULE


def kernel(x, noise_var, noise_bias):
    x = np.ascontiguousarray(np.asarray(x, dtype=np.float32))
    nv = float(np.asarray(noise_var).reshape(-1)[0])
    nb = float(np.asarray(noise_bias).reshape(-1)[0])
    B = x.shape[0]
    assert x.shape == (B, 1, H, W) and B == N_CORES * IMGS_PER_CORE

    nvb = np.empty((128, 2), np.float32)
    nvb[:, 0] = nv
    nvb[:, 1] = nb

    nc = _get_module()
    xpad = np.zeros((B, H + 4, WIDE), np.float32)
    xpad[:, 2:2 + H, 2:2 + W] = x[:, 0]
    in_maps = []
    for c in range(N_CORES):
        shard = np.ascontiguousarray(
            xpad[c * IMGS_PER_CORE:(c + 1) * IMGS_PER_CORE])
        in_maps.append({"x": shard, "nvb": nvb})

    res = run_bass_kernel_spmd(nc, in_maps, core_ids=list(range(N_CORES)))
    y = np.empty((B, 1, H, W), np.float32)
    for c in range(N_CORES):
        y[c * IMGS_PER_CORE:(c + 1) * IMGS_PER_CORE, 0] = res.results[c]["y"]
    return y



# revision 71
# speedup vs baseline: 1.6697x; 1.6697x over previous
"""Trainium2 Bass kernel: 5x5 window median+variance denoise filter.

y = relu(x - noise_var/(var5x5(x)+1e-10) * (x - median5x5(x) + noise_bias))
with zero-padded 5x5 windows, unbiased variance (ddof=1).

Sharding: pure data parallel, B=16 images split 2-per-core across 8 cores.

Median via a pruned comparator network with shared column sorts:
  sort5 over the 5 dy-shifted planes (9 CE, shared by 5 horizontal windows)
  T = odd-even merge of adjacent sorted columns (13 CE, shared by 2 windows)
  final rank-12 selection from T(x-2), T(x), S(x+2) (35 CE, single-sided
  min/max pruned) -- 90 DVE min/max ops per full-image sweep, verified
  offline by exhaustive 0-1 principle.
"""
import numpy as np

import concourse.bass as bass  # noqa: F401
import concourse.mybir as mybir
from concourse import bacc, tile
from concourse.bass_utils import run_bass_kernel_spmd

F32 = mybir.dt.float32
FP16 = mybir.dt.float16
ALU = mybir.AluOpType
ACTF = mybir.ActivationFunctionType
MEDIAN_FP16 = True    # fp16 comparator network (first sort layer does the
                      # f32->fp16 cast for free; DVE-legal, ~5e-4 rel err)

# (i, j, need_min, need_max) per structure; designed + 0/1-verified offline.
SORT5 = [(0, 1, 1, 1), (3, 4, 1, 1), (2, 4, 1, 1), (2, 3, 1, 1), (0, 3, 1, 1),
         (0, 2, 1, 1), (1, 4, 1, 1), (1, 3, 1, 1), (1, 2, 1, 1)]
T_CES = [(0, 5, 1, 1), (4, 9, 1, 1), (4, 5, 1, 1), (2, 7, 1, 1), (2, 4, 1, 1),
         (7, 5, 1, 1), (1, 6, 1, 1), (3, 8, 1, 1), (3, 6, 1, 1), (1, 2, 1, 1),
         (3, 4, 1, 1), (6, 7, 1, 1), (8, 5, 1, 1)]
F_CES = [(0, 10, 0, 1), (5, 15, 1, 0), (5, 10, 1, 1), (4, 14, 1, 1),
         (4, 5, 0, 1), (14, 10, 1, 0), (2, 12, 0, 1), (7, 17, 1, 0),
         (7, 12, 1, 1), (7, 5, 0, 1), (12, 14, 1, 1), (1, 11, 0, 1),
         (9, 19, 1, 0), (9, 11, 1, 1), (6, 16, 1, 1), (6, 9, 0, 1),
         (16, 11, 1, 0), (3, 13, 0, 1), (8, 18, 1, 0), (8, 13, 1, 1),
         (8, 9, 1, 1), (13, 16, 1, 0), (8, 5, 1, 1), (9, 12, 1, 1),
         (13, 14, 1, 1), (8, 20, 0, 1), (13, 24, 1, 0), (13, 20, 0, 1),
         (9, 22, 0, 1), (22, 20, 1, 0), (5, 21, 0, 1), (14, 21, 1, 0),
         (12, 23, 1, 0), (12, 14, 0, 1), (14, 22, 1, 0)]
F_OUT = 14

H = 512
W = 512
IMGS_PER_CORE = 2
N_CORES = 8
WIDE = W + 4          # 2-col halo each side
NBUF = 46             # cap on SBUF working buffers of [128, 2, WIDE] f32


class BufPool:
    """Free-list over preallocated fixed SBUF tensors. Tile's dependency
    tracker makes reuse safe (WAR/RAW serialization on the same tensor)."""

    def __init__(self, nc, tag="wb", dtype=F32, cap=NBUF):
        self.nc = nc
        self.tag = tag
        self.dtype = dtype
        self.cap = cap
        self.bufs = []
        self.free = []

    def alloc(self):
        if self.free:
            return self.free.pop()
        idx = len(self.bufs)
        assert idx < self.cap, f"SBUF pool {self.tag} exhausted"
        t = self.nc.alloc_sbuf_tensor(f"{self.tag}{idx}", [128, 2, WIDE],
                                      self.dtype).ap()
        self.bufs.append(t)
        return t

    def release(self, t):
        self.free.append(t)


class Wire:
    """SSA value living at column offset `off` of `buf`."""

    def __init__(self, buf, off, owned, pool, on_die=None):
        self.buf = buf
        self.off = off
        self.owned = owned      # release buf to pool when dead
        self.pool = pool
        self.reads_left = 0
        self.on_die = on_die

    def ap(self, width):
        return self.buf[:, :, self.off:self.off + width]

    def read_done(self):
        self.reads_left -= 1
        if self.reads_left == 0:
            self._die()

    def read_done_zero(self):
        if self.reads_left == 0:
            self._die()

    def _die(self):
        if self.owned:
            self.pool.release(self.buf)
        if self.on_die is not None:
            self.on_die()

    def detach_views(self, n_views):
        """Transfer buffer ownership to n_views future views; returns the
        on_die callback the views share. Call read_done() after to consume
        the terminal hold."""
        buf, owned, pool = self.buf, self.owned, self.pool
        self.owned = False
        state = {"n": n_views}

        def on_die():
            state["n"] -= 1
            if state["n"] == 0 and owned:
                pool.release(buf)
        return on_die


def run_stage(nc, pool, wires, ces, width, terminal_reads, final_pool=None,
              final_wire=None):
    """Emit one structure stage. A position's lifetime is split into segments
    at each rewrite; each Wire object gets the read count of its own segment
    so buffers release as soon as truly dead. The final write of
    `final_wire` (if given) allocates from `final_pool` (dtype switch)."""
    n = len(wires)
    # segs[i] = read counts per segment of position i (segment ends at a
    # write of i, which itself reads the old value).
    segs = [[] for _ in range(n)]
    cur = [0] * n
    last_write = {}
    for ci, (a, b, nmin, nmax) in enumerate(ces):
        cur[a] += 1
        cur[b] += 1
        if nmin:
            segs[a].append(cur[a])
            cur[a] = 0
            last_write[a] = (ci, "min")
        if nmax:
            segs[b].append(cur[b])
            cur[b] = 0
            last_write[b] = (ci, "max")
    for i in range(n):
        segs[i].append(cur[i] + terminal_reads.get(i, 0))

    seg_idx = [0] * n
    for i in range(n):
        wires[i].reads_left += segs[i][0]
        if segs[i][0] == 0:
            wires[i].read_done_zero()

    for ci, (i, j, nmin, nmax) in enumerate(ces):
        wi, wj = wires[i], wires[j]
        a = wi.ap(width)
        b = wj.ap(width)
        if nmin:
            p = (final_pool if final_wire == i
                 and last_write.get(i) == (ci, "min") else pool)
            lo_pool = p
            lo = p.alloc()
            nc.vector.tensor_tensor(lo[:, :, 0:width], a, b, ALU.min)
        if nmax:
            p = (final_pool if final_wire == j
                 and last_write.get(j) == (ci, "max") else pool)
            hi_pool = p
            hi = p.alloc()
            nc.vector.tensor_tensor(hi[:, :, 0:width], a, b, ALU.max)
        wi.read_done()
        wj.read_done()
        if nmin:
            seg_idx[i] += 1
            cnt = segs[i][seg_idx[i]]
            assert cnt > 0, "dead write (should be pruned offline)"
            wires[i] = Wire(lo, 0, True, lo_pool)
            wires[i].reads_left = cnt
        if nmax:
            seg_idx[j] += 1
            cnt = segs[j][seg_idx[j]]
            assert cnt > 0, "dead write (should be pruned offline)"
            wires[j] = Wire(hi, 0, True, hi_pool)
            wires[j].reads_left = cnt


def emit_chunk(nc, pool, hpool, tin, out_tile, xa, ya, nv_ap, nb_ap, img,
               half):
    r0 = half * 256

    # ---- loads: 5 dy-shifted tiles [128, 2, WIDE] from the pre-padded
    # shard (rows/cols already carry the 2-wide zero halo) ----
    for k, dy in enumerate(range(-2, 3)):
        for b in range(2):
            s = img * (H + 4) + r0 + b * 128 + dy + 2
            nc.sync.dma_start(tin[k][:, b, :], xa[s: s + 128, :])

    # ---- variance ----
    full = lambda t: t[:, :, :]
    sum_acc = pool.alloc()
    nc.vector.tensor_tensor(full(sum_acc), full(tin[0]), full(tin[1]), ALU.add)
    for k in (2, 3, 4):
        nc.vector.tensor_tensor(full(sum_acc), full(sum_acc), full(tin[k]),
                                ALU.add)
    sq_acc = pool.alloc()
    nc.scalar.square(full(sq_acc), full(tin[0]))
    # two alternating temps so ACT's square k+1 never WAR-waits on DVE's
    # add of square k
    sq_tmps = [pool.alloc(), pool.alloc()]
    for k in (1, 2, 3, 4):
        sq_tmp = sq_tmps[k % 2]
        nc.scalar.square(full(sq_tmp), full(tin[k]))
        nc.vector.tensor_tensor(full(sq_acc), full(sq_acc), full(sq_tmp),
                                ALU.add)
    pool.release(sq_tmps[0])
    pool.release(sq_tmps[1])

    def hsum(acc, eps=None):
        # 3-op tree instead of 4 serial adds; eps (folded 24*1e-10) rides
        # the last op's stt scalar slot
        t1 = pool.alloc()
        nc.vector.tensor_tensor(t1[:, :, 0:W + 3], acc[:, :, 0:W + 3],
                                acc[:, :, 1:W + 4], ALU.add)
        t2 = pool.alloc()
        nc.vector.tensor_tensor(t2[:, :, 0:W + 1], t1[:, :, 0:W + 1],
                                t1[:, :, 2:W + 3], ALU.add)
        pool.release(t1)
        o = pool.alloc()
        if eps is None:
            nc.vector.tensor_tensor(o[:, :, 0:W], t2[:, :, 0:W],
                                    acc[:, :, 4:W + 4], ALU.add)
        else:
            nc.vector.scalar_tensor_tensor(o[:, :, 0:W], t2[:, :, 0:W], eps,
                                           acc[:, :, 4:W + 4], ALU.add,
                                           ALU.add)
        pool.release(t2)
        pool.release(acc)
        return o

    s25 = hsum(sum_acc)
    q25 = hsum(sq_acc, eps=24e-10)

    d = pool.alloc()
    nc.vector.tensor_tensor(d[:, :, 0:W], s25[:, :, 0:W], s25[:, :, 0:W],
                            ALU.mult)
    pool.release(s25)
    # d = (s25^2 * (-1/25)) + q25 = 24*var + 24e-10 (1/24 folded into the
    # host-side -24*nv scalar, eps folded into q25 above)
    nc.vector.scalar_tensor_tensor(d[:, :, 0:W], d[:, :, 0:W], -1.0 / 25.0,
                                   q25[:, :, 0:W], ALU.mult, ALU.add)
    pool.release(q25)
    rcp = pool.alloc()
    # single-op ~51-ULP reciprocal (HW-verified 3e-6 rel err) vs the 2-op
    # approx_accurate the first version used
    nc.vector.reciprocal_approx_fast(rcp[:, :, 0:W], d[:, :, 0:W])
    pool.release(d)

    # ---- median network (fp16 when MEDIAN_FP16: the first sort layer
    # reads the f32 tiles and writes fp16; final F op emits f32) ----
    np_ = hpool if MEDIAN_FP16 else pool
    s_wires = [Wire(tin[k], 0, False, pool) for k in range(5)]
    run_stage(nc, np_, s_wires, SORT5, WIDE, {k: 1 for k in range(5)})

    t_wires = [None] * 10
    c_views = [None] * 5
    for k in range(5):
        rk = s_wires[k]
        od = rk.detach_views(3)
        t_wires[k] = Wire(rk.buf, rk.off + 0, False, np_, on_die=od)
        t_wires[k + 5] = Wire(rk.buf, rk.off + 1, False, np_, on_die=od)
        c_views[k] = Wire(rk.buf, rk.off + 4, False, np_, on_die=od)
        rk.read_done()      # consume terminal hold

    run_stage(nc, np_, t_wires, T_CES, W + 3, {j: 1 for j in range(10)})

    f_wires = [None] * 25
    for j in range(10):
        tw = t_wires[j]
        od = tw.detach_views(2)
        f_wires[j] = Wire(tw.buf, tw.off + 0, False, np_, on_die=od)
        f_wires[j + 10] = Wire(tw.buf, tw.off + 2, False, np_, on_die=od)
        tw.read_done()
    for k in range(5):
        f_wires[20 + k] = c_views[k]

    run_stage(nc, np_, f_wires, F_CES, W, {F_OUT: 1},
              final_pool=pool, final_wire=F_OUT)
    mid = f_wires[F_OUT]

    # ---- formula: y = relu(x + (-24nv)*rcp*((x + nb) - mid)) ----
    xc = tin[2][:, :, 2:2 + W]              # center plane = x
    u = pool.alloc()
    nc.vector.scalar_tensor_tensor(u[:, :, 0:W], xc, nb_ap, mid.ap(W),
                                   ALU.add, ALU.subtract)
    mid.read_done()
    nc.vector.tensor_tensor(u[:, :, 0:W], rcp[:, :, 0:W], u[:, :, 0:W],
                            ALU.mult)
    pool.release(rcp)
    # y' = (u * -24nv) + x in one stt (nv_ap holds -24*noise_var)
    nc.vector.scalar_tensor_tensor(u[:, :, 0:W], u[:, :, 0:W], nv_ap, xc,
                                   ALU.mult, ALU.add)
    nc.scalar.activation(out_tile[:, :, :], u[:, :, 0:W], ACTF.Relu)
    pool.release(u)

    # ---- store ----
    for b in range(2):
        nc.sync.dma_start(
            ya[img * H + r0 + b * 128: img * H + r0 + b * 128 + 128, :],
            out_tile[:, b, :],
        )


def build_module(repeat=1, hw_loop=None):
    nc = bacc.Bacc(
        "TRN2",
        target_bir_lowering=False,
        debug=False,
        enable_asserts=False,
        num_devices=N_CORES,
    )
    x = nc.dram_tensor("x", [IMGS_PER_CORE, H + 4, WIDE], F32,
                       kind="ExternalInput")
    nvb = nc.dram_tensor("nvb", [128, 2], F32, kind="ExternalInput")
    y = nc.dram_tensor("y", [IMGS_PER_CORE, H, W], F32, kind="ExternalOutput")

    xa = x.ap().flatten_outer_dims()    # [2*516, 516]
    ya = y.ap().flatten_outer_dims()

    with tile.TileContext(nc) as tc:
        pool = BufPool(nc, "wb", F32, 18 if MEDIAN_FP16 else NBUF)
        hpool = BufPool(nc, "wh", FP16, 40)
        nvb_t = nc.alloc_sbuf_tensor("nvb_t", [128, 2], F32).ap()
        nc.sync.dma_start(nvb_t[:, :], nvb.ap()[:, :])
        nv_ap = nvb_t[:, 0:1]
        nb_ap = nvb_t[:, 1:2]

        tin = [nc.alloc_sbuf_tensor(f"tin{k}", [128, 2, WIDE], F32).ap()
               for k in range(5)]
        out_t = [nc.alloc_sbuf_tensor(f"out{k}", [128, 2, W], F32).ap()
                 for k in range(2)]

        def body():
            for _ in range(repeat):
                for img in range(IMGS_PER_CORE):
                    for half in range(2):
                        emit_chunk(nc, pool, hpool, tin, out_t[half], xa, ya,
                                   nv_ap, nb_ap, img, half)

        if hw_loop is None:
            body()
        else:
            with tc.For_i(0, hw_loop, 1):
                body()

    nc.compile()
    return nc


_MODULE = None


def _get_module():
    global _MODULE
    if _MODULE is None:
        _MODULE = build_module()
    return _MODULE


def kernel(x, noise_var, noise_bias):
    x = np.ascontiguousarray(np.asarray(x, dtype=np.float32))
    nv = float(np.asarray(noise_var).reshape(-1)[0])
    nb = float(np.asarray(noise_bias).reshape(-1)[0])
    B = x.shape[0]
    assert x.shape == (B, 1, H, W) and B == N_CORES * IMGS_PER_CORE

    nvb = np.empty((128, 2), np.float32)
    nvb[:, 0] = -24.0 * nv   # 1/24 ddof factor folded in
    nvb[:, 1] = nb

    nc = _get_module()
    xpad = np.zeros((B, H + 4, WIDE), np.float32)
    xpad[:, 2:2 + H, 2:2 + W] = x[:, 0]
    in_maps = []
    for c in range(N_CORES):
        shard = np.ascontiguousarray(
            xpad[c * IMGS_PER_CORE:(c + 1) * IMGS_PER_CORE])
        in_maps.append({"x": shard, "nvb": nvb})

    res = run_bass_kernel_spmd(nc, in_maps, core_ids=list(range(N_CORES)))
    y = np.empty((B, 1, H, W), np.float32)
    for c in range(N_CORES):
        y[c * IMGS_PER_CORE:(c + 1) * IMGS_PER_CORE, 0] = res.results[c]["y"]
    return y



# revision 73
# speedup vs baseline: 2.3146x; 1.3862x over previous
"""Trainium2 Bass kernel: 5x5 window median+variance denoise filter.

y = relu(x - noise_var/(var5x5(x)+1e-10) * (x - median5x5(x) + noise_bias))
with zero-padded 5x5 windows, unbiased variance (ddof=1).

Sharding: pure data parallel, B=16 images split 2-per-core across 8 cores.

Median via a pruned comparator network with shared column sorts:
  sort5 over the 5 dy-shifted planes (9 CE, shared by 5 horizontal windows)
  T = odd-even merge of adjacent sorted columns (13 CE, shared by 2 windows)
  final rank-12 selection from T(x-2), T(x), S(x+2) (35 CE, single-sided
  min/max pruned) -- 90 DVE min/max ops per full-image sweep, verified
  offline by exhaustive 0-1 principle.
"""
import numpy as np

import concourse.bass as bass  # noqa: F401
import concourse.mybir as mybir
from concourse import bacc, tile
from concourse.bass_utils import run_bass_kernel_spmd

F32 = mybir.dt.float32
FP16 = mybir.dt.float16
ALU = mybir.AluOpType
ACTF = mybir.ActivationFunctionType
MEDIAN_FP16 = True    # fp16 comparator network (first sort layer does the
                      # f32->fp16 cast for free; DVE-legal, ~5e-4 rel err)

# (i, j, need_min, need_max) per structure; designed + 0/1-verified offline.
SORT5 = [(0, 1, 1, 1), (3, 4, 1, 1), (2, 4, 1, 1), (2, 3, 1, 1), (0, 3, 1, 1),
         (0, 2, 1, 1), (1, 4, 1, 1), (1, 3, 1, 1), (1, 2, 1, 1)]
T_CES = [(0, 5, 1, 1), (4, 9, 1, 1), (4, 5, 1, 1), (2, 7, 1, 1), (2, 4, 1, 1),
         (7, 5, 1, 1), (1, 6, 1, 1), (3, 8, 1, 1), (3, 6, 1, 1), (1, 2, 1, 1),
         (3, 4, 1, 1), (6, 7, 1, 1), (8, 5, 1, 1)]
F_CES = [(0, 10, 0, 1), (5, 15, 1, 0), (5, 10, 1, 1), (4, 14, 1, 1),
         (4, 5, 0, 1), (14, 10, 1, 0), (2, 12, 0, 1), (7, 17, 1, 0),
         (7, 12, 1, 1), (7, 5, 0, 1), (12, 14, 1, 1), (1, 11, 0, 1),
         (9, 19, 1, 0), (9, 11, 1, 1), (6, 16, 1, 1), (6, 9, 0, 1),
         (16, 11, 1, 0), (3, 13, 0, 1), (8, 18, 1, 0), (8, 13, 1, 1),
         (8, 9, 1, 1), (13, 16, 1, 0), (8, 5, 1, 1), (9, 12, 1, 1),
         (13, 14, 1, 1), (8, 20, 0, 1), (13, 24, 1, 0), (13, 20, 0, 1),
         (9, 22, 0, 1), (22, 20, 1, 0), (5, 21, 0, 1), (14, 21, 1, 0),
         (12, 23, 1, 0), (12, 14, 0, 1), (14, 22, 1, 0)]
F_OUT = 14

H = 512
W = 512
IMGS_PER_CORE = 2
N_CORES = 8
WIDE = W + 4          # 2-col halo each side
NBUF = 46             # cap on SBUF working buffers of [128, 2, WIDE] f32


class BufPool:
    """Free-list over preallocated fixed SBUF tensors. Tile's dependency
    tracker makes reuse safe (WAR/RAW serialization on the same tensor)."""

    def __init__(self, nc, tag="wb", dtype=F32, cap=NBUF):
        self.nc = nc
        self.tag = tag
        self.dtype = dtype
        self.cap = cap
        self.bufs = []
        self.free = []

    def alloc(self):
        if self.free:
            return self.free.pop()
        idx = len(self.bufs)
        assert idx < self.cap, f"SBUF pool {self.tag} exhausted"
        t = self.nc.alloc_sbuf_tensor(f"{self.tag}{idx}", [128, 2, WIDE],
                                      self.dtype).ap()
        self.bufs.append(t)
        return t

    def release(self, t):
        self.free.append(t)


class Wire:
    """SSA value living at column offset `off` of `buf`."""

    def __init__(self, buf, off, owned, pool, on_die=None):
        self.buf = buf
        self.off = off
        self.owned = owned      # release buf to pool when dead
        self.pool = pool
        self.reads_left = 0
        self.on_die = on_die

    def ap(self, width):
        return self.buf[:, :, self.off:self.off + width]

    def read_done(self):
        self.reads_left -= 1
        if self.reads_left == 0:
            self._die()

    def read_done_zero(self):
        if self.reads_left == 0:
            self._die()

    def _die(self):
        if self.owned:
            self.pool.release(self.buf)
        if self.on_die is not None:
            self.on_die()

    def detach_views(self, n_views):
        """Transfer buffer ownership to n_views future views; returns the
        on_die callback the views share. Call read_done() after to consume
        the terminal hold."""
        buf, owned, pool = self.buf, self.owned, self.pool
        self.owned = False
        state = {"n": n_views}

        def on_die():
            state["n"] -= 1
            if state["n"] == 0 and owned:
                pool.release(buf)
        return on_die


def run_stage(nc, pool, wires, ces, width, terminal_reads, final_pool=None,
              final_wire=None):
    """Emit one structure stage. A position's lifetime is split into segments
    at each rewrite; each Wire object gets the read count of its own segment
    so buffers release as soon as truly dead. The final write of
    `final_wire` (if given) allocates from `final_pool` (dtype switch)."""
    n = len(wires)
    # segs[i] = read counts per segment of position i (segment ends at a
    # write of i, which itself reads the old value).
    segs = [[] for _ in range(n)]
    cur = [0] * n
    last_write = {}
    for ci, (a, b, nmin, nmax) in enumerate(ces):
        cur[a] += 1
        cur[b] += 1
        if nmin:
            segs[a].append(cur[a])
            cur[a] = 0
            last_write[a] = (ci, "min")
        if nmax:
            segs[b].append(cur[b])
            cur[b] = 0
            last_write[b] = (ci, "max")
    for i in range(n):
        segs[i].append(cur[i] + terminal_reads.get(i, 0))

    seg_idx = [0] * n
    for i in range(n):
        wires[i].reads_left += segs[i][0]
        if segs[i][0] == 0:
            wires[i].read_done_zero()

    for ci, (i, j, nmin, nmax) in enumerate(ces):
        wi, wj = wires[i], wires[j]
        a = wi.ap(width)
        b = wj.ap(width)
        if nmin:
            p = (final_pool if final_wire == i
                 and last_write.get(i) == (ci, "min") else pool)
            lo_pool = p
            lo = p.alloc()
            nc.vector.tensor_tensor(lo[:, :, 0:width], a, b, ALU.min)
        if nmax:
            p = (final_pool if final_wire == j
                 and last_write.get(j) == (ci, "max") else pool)
            hi_pool = p
            hi = p.alloc()
            nc.vector.tensor_tensor(hi[:, :, 0:width], a, b, ALU.max)
        wi.read_done()
        wj.read_done()
        if nmin:
            seg_idx[i] += 1
            cnt = segs[i][seg_idx[i]]
            assert cnt > 0, "dead write (should be pruned offline)"
            wires[i] = Wire(lo, 0, True, lo_pool)
            wires[i].reads_left = cnt
        if nmax:
            seg_idx[j] += 1
            cnt = segs[j][seg_idx[j]]
            assert cnt > 0, "dead write (should be pruned offline)"
            wires[j] = Wire(hi, 0, True, hi_pool)
            wires[j].reads_left = cnt


def emit_chunk(nc, pool, hpool, tin, out_tile, xa, ya, nv_ap, nb_ap, img,
               half):
    r0 = half * 256

    # ---- loads: 5 dy-shifted tiles [128, 2, WIDE] from the pre-padded
    # shard (rows/cols already carry the 2-wide zero halo) ----
    for k, dy in enumerate(range(-2, 3)):
        for b in range(2):
            s = img * (H + 4) + r0 + b * 128 + dy + 2
            nc.sync.dma_start(tin[k][:, b, :], xa[s: s + 128, :])

    # ---- variance ----
    full = lambda t: t[:, :, :]
    sum_acc = pool.alloc()
    nc.vector.tensor_tensor(full(sum_acc), full(tin[0]), full(tin[1]), ALU.add)
    for k in (2, 3, 4):
        nc.vector.tensor_tensor(full(sum_acc), full(sum_acc), full(tin[k]),
                                ALU.add)
    sq_acc = pool.alloc()
    nc.scalar.square(full(sq_acc), full(tin[0]))
    # two alternating temps so ACT's square k+1 never WAR-waits on DVE's
    # add of square k
    sq_tmps = [pool.alloc(), pool.alloc()]
    for k in (1, 2, 3, 4):
        sq_tmp = sq_tmps[k % 2]
        nc.scalar.square(full(sq_tmp), full(tin[k]))
        nc.vector.tensor_tensor(full(sq_acc), full(sq_acc), full(sq_tmp),
                                ALU.add)
    pool.release(sq_tmps[0])
    pool.release(sq_tmps[1])

    def hsum(acc, eps=None):
        # 3-op tree instead of 4 serial adds; eps (folded 24*1e-10) rides
        # the last op's stt scalar slot
        t1 = pool.alloc()
        nc.vector.tensor_tensor(t1[:, :, 0:W + 3], acc[:, :, 0:W + 3],
                                acc[:, :, 1:W + 4], ALU.add)
        t2 = pool.alloc()
        nc.vector.tensor_tensor(t2[:, :, 0:W + 1], t1[:, :, 0:W + 1],
                                t1[:, :, 2:W + 3], ALU.add)
        pool.release(t1)
        o = pool.alloc()
        if eps is None:
            nc.vector.tensor_tensor(o[:, :, 0:W], t2[:, :, 0:W],
                                    acc[:, :, 4:W + 4], ALU.add)
        else:
            nc.vector.scalar_tensor_tensor(o[:, :, 0:W], t2[:, :, 0:W], eps,
                                           acc[:, :, 4:W + 4], ALU.add,
                                           ALU.add)
        pool.release(t2)
        pool.release(acc)
        return o

    s25 = hsum(sum_acc)
    q25 = hsum(sq_acc, eps=24e-10)

    d = pool.alloc()
    nc.vector.tensor_tensor(d[:, :, 0:W], s25[:, :, 0:W], s25[:, :, 0:W],
                            ALU.mult)
    pool.release(s25)
    # d = (s25^2 * (-1/25)) + q25 = 24*var + 24e-10 (1/24 folded into the
    # host-side -24*nv scalar, eps folded into q25 above)
    nc.vector.scalar_tensor_tensor(d[:, :, 0:W], d[:, :, 0:W], -1.0 / 25.0,
                                   q25[:, :, 0:W], ALU.mult, ALU.add)
    pool.release(q25)
    rcp = pool.alloc()
    # single-op ~51-ULP reciprocal (HW-verified 3e-6 rel err) vs the 2-op
    # approx_accurate the first version used
    nc.vector.reciprocal_approx_fast(rcp[:, :, 0:W], d[:, :, 0:W])
    pool.release(d)

    # ---- median network (fp16 when MEDIAN_FP16: the first sort layer
    # reads the f32 tiles and writes fp16; final F op emits f32) ----
    np_ = hpool if MEDIAN_FP16 else pool
    s_wires = [Wire(tin[k], 0, False, pool) for k in range(5)]
    run_stage(nc, np_, s_wires, SORT5, WIDE, {k: 1 for k in range(5)})

    t_wires = [None] * 10
    c_views = [None] * 5
    for k in range(5):
        rk = s_wires[k]
        od = rk.detach_views(3)
        t_wires[k] = Wire(rk.buf, rk.off + 0, False, np_, on_die=od)
        t_wires[k + 5] = Wire(rk.buf, rk.off + 1, False, np_, on_die=od)
        c_views[k] = Wire(rk.buf, rk.off + 4, False, np_, on_die=od)
        rk.read_done()      # consume terminal hold

    run_stage(nc, np_, t_wires, T_CES, W + 3, {j: 1 for j in range(10)})

    f_wires = [None] * 25
    for j in range(10):
        tw = t_wires[j]
        od = tw.detach_views(2)
        f_wires[j] = Wire(tw.buf, tw.off + 0, False, np_, on_die=od)
        f_wires[j + 10] = Wire(tw.buf, tw.off + 2, False, np_, on_die=od)
        tw.read_done()
    for k in range(5):
        f_wires[20 + k] = c_views[k]

    run_stage(nc, np_, f_wires, F_CES, W, {F_OUT: 1},
              final_pool=pool, final_wire=F_OUT)
    mid = f_wires[F_OUT]

    # ---- formula: y = relu(x + (-24nv)*rcp*((x + nb) - mid)) ----
    xc = tin[2][:, :, 2:2 + W]              # center plane = x
    u = pool.alloc()
    nc.vector.scalar_tensor_tensor(u[:, :, 0:W], xc, nb_ap, mid.ap(W),
                                   ALU.add, ALU.subtract)
    mid.read_done()
    nc.vector.tensor_tensor(u[:, :, 0:W], rcp[:, :, 0:W], u[:, :, 0:W],
                            ALU.mult)
    pool.release(rcp)
    # y' = (u * -24nv) + x in one stt (nv_ap holds -24*noise_var)
    nc.vector.scalar_tensor_tensor(u[:, :, 0:W], u[:, :, 0:W], nv_ap, xc,
                                   ALU.mult, ALU.add)
    nc.scalar.activation(out_tile[:, :, :], u[:, :, 0:W], ACTF.Relu)
    pool.release(u)

    # ---- store ----
    for b in range(2):
        nc.sync.dma_start(
            ya[img * H + r0 + b * 128: img * H + r0 + b * 128 + 128, :],
            out_tile[:, b, :],
        )


def build_module(repeat=1, hw_loop=None):
    nc = bacc.Bacc(
        "TRN2",
        target_bir_lowering=False,
        debug=False,
        enable_asserts=False,
        num_devices=N_CORES,
    )
    x = nc.dram_tensor("x", [IMGS_PER_CORE, H + 4, WIDE], F32,
                       kind="ExternalInput")
    nvb = nc.dram_tensor("nvb", [128, 2], F32, kind="ExternalInput")
    y = nc.dram_tensor("y", [IMGS_PER_CORE, H, W], F32, kind="ExternalOutput")

    xa = x.ap().flatten_outer_dims()    # [2*516, 516]
    ya = y.ap().flatten_outer_dims()

    with tile.TileContext(nc) as tc:
        pool = BufPool(nc, "wb", F32, 14 if MEDIAN_FP16 else NBUF)
        hpool = BufPool(nc, "wh", FP16, 40)
        nvb_t = nc.alloc_sbuf_tensor("nvb_t", [128, 2], F32).ap()
        nc.sync.dma_start(nvb_t[:, :], nvb.ap()[:, :])
        nv_ap = nvb_t[:, 0:1]
        nb_ap = nvb_t[:, 1:2]

        # double-buffered input tiles: chunk c+1's loads no longer WAR-wait
        # on chunk c's last tin reader (the formula reads tin[2] at chunk
        # end), so DMA prefetch overlaps compute
        tins = [[nc.alloc_sbuf_tensor(f"tin{p}_{k}", [128, 2, WIDE],
                                      F32).ap() for k in range(5)]
                for p in range(2)]
        out_t = [nc.alloc_sbuf_tensor(f"out{k}", [128, 2, W], F32).ap()
                 for k in range(2)]

        def body():
            ci = 0
            for _ in range(repeat):
                for img in range(IMGS_PER_CORE):
                    for half in range(2):
                        emit_chunk(nc, pool, hpool, tins[ci % 2],
                                   out_t[half], xa, ya, nv_ap, nb_ap, img,
                                   half)
                        ci += 1

        if hw_loop is None:
            body()
        else:
            with tc.For_i(0, hw_loop, 1):
                body()

    nc.compile()
    return nc


_MODULE = None


def _get_module():
    global _MODULE
    if _MODULE is None:
        _MODULE = build_module()
    return _MODULE


def kernel(x, noise_var, noise_bias):
    x = np.ascontiguousarray(np.asarray(x, dtype=np.float32))
    nv = float(np.asarray(noise_var).reshape(-1)[0])
    nb = float(np.asarray(noise_bias).reshape(-1)[0])
    B = x.shape[0]
    assert x.shape == (B, 1, H, W) and B == N_CORES * IMGS_PER_CORE

    nvb = np.empty((128, 2), np.float32)
    nvb[:, 0] = -24.0 * nv   # 1/24 ddof factor folded in
    nvb[:, 1] = nb

    nc = _get_module()
    xpad = np.zeros((B, H + 4, WIDE), np.float32)
    xpad[:, 2:2 + H, 2:2 + W] = x[:, 0]
    in_maps = []
    for c in range(N_CORES):
        shard = np.ascontiguousarray(
            xpad[c * IMGS_PER_CORE:(c + 1) * IMGS_PER_CORE])
        in_maps.append({"x": shard, "nvb": nvb})

    res = run_bass_kernel_spmd(nc, in_maps, core_ids=list(range(N_CORES)))
    y = np.empty((B, 1, H, W), np.float32)
    for c in range(N_CORES):
        y[c * IMGS_PER_CORE:(c + 1) * IMGS_PER_CORE, 0] = res.results[c]["y"]
    return y

